# revision 1
# baseline (speedup 1.0000x reference)
"""Causal multi-head attention block on 8 Trainium2 NeuronCores.

Problem (hardcoded): bs=2, n_ctx=2048, d_model=1024, 16 heads, dk=dv=64.
Sharding: core = (batch b, head-group g of 4 heads); b = core//4, g = core%4.
Each core computes y_partial[b] = Attn(x[b], heads 4g..4g+3) @ Wo[:, 256g:256(g+1)].T
Host sums the 4 partials per batch. Biases are zero in this problem and skipped.

Device layout choices:
  - x is fed pre-transposed (xT = x[b].T) and in bf16 so d_model lands on
    partitions for every projection matmul (PE contracts over partitions).
  - Q,K are produced transposed (QT/KT = [2*64 head-pair dims, n]); scores are
    computed in S.T layout [keys, q] so softmax probs P.T are directly the
    moving operand for PV, with V row-major [keys, dv] as the stationary one.
  - V carries an appended ones column, so PV ([V|1].T @ P.T) emits the softmax
    denominator as row 64 of the PSUM tile; normalization happens during PSUM
    eviction (reciprocal + broadcast multiply).
  - Causality: key-tiles fully above the diagonal are skipped; the 4 diagonal
    128x128 blocks per 512-wide q-chunk get a triangular 0/1 mask after exp.
"""

import sys
import numpy as np

sys.path.insert(0, "/opt/trn_rl_repo")

import ml_dtypes

import concourse.bass as bass
import concourse.mybir as mybir
import concourse.tile as tile
from concourse import bacc
from concourse.bass_utils import run_bass_kernel_spmd

BF16 = ml_dtypes.bfloat16
F32 = mybir.dt.float32
BF = mybir.dt.bfloat16

BS, N, DM = 2, 2048, 1024
H_TOT, DK = 16, 64
HPC = 4           # heads per core
PAIRS = 2         # head pairs per core (2 heads of 64 share 128 partitions)
NC_CORES = 8
QC = 512          # q-chunk width
KT = 128          # key tile
NQC = N // QC     # 4
NKT = N // KT     # 16
CCH = DM // 128   # 8 contraction chunks for projections


def _bcast_part(ap, nparts):
    """Broadcast a 1-partition AP across nparts partitions (step-0 AP)."""
    return bass.AP(tensor=ap.tensor, offset=ap.offset, ap=[[0, nparts]] + list(ap.ap)[1:])


def _free_repeat(ap, repeat):
    """Insert a step-0 free dim: [P, k] -> [P, repeat, k]."""
    a = list(ap.ap)
    return bass.AP(tensor=ap.tensor, offset=ap.offset, ap=[a[0], [0, repeat]] + a[1:])


def build_program(parts="full"):
    nc = bacc.Bacc(
        "TRN2",
        target_bir_lowering=False,
        debug=False,
        enable_asserts=False,
        num_devices=NC_CORES,
    )
    xT = nc.dram_tensor("xT", (DM, N), BF, kind="ExternalInput").ap()
    wqT = nc.dram_tensor("wqT", (DM, 256), BF, kind="ExternalInput").ap()
    wkT = nc.dram_tensor("wkT", (DM, 256), BF, kind="ExternalInput").ap()
    wvT = nc.dram_tensor("wvT", (DM, 256), BF, kind="ExternalInput").ap()
    woT = nc.dram_tensor("woT", (256, DM), BF, kind="ExternalInput").ap()
    tri = nc.dram_tensor("tri", (128, 128), BF, kind="ExternalInput").ap()
    y = nc.dram_tensor("y", (N, DM), F32, kind="ExternalOutput").ap()
    rc_d = nc.dram_tensor("rc_scratch", (NQC * PAIRS, 1024), F32).ap()

    with tile.TileContext(nc) as tc:
        _emit(nc, tc, xT, wqT, wkT, wvT, woT, tri, y, rc_d, parts)
    nc.compile()
    return nc


def _emit(nc, tc, xT, wqT, wkT, wvT, woT, tri, y, rc_d, parts="full"):
    from collections import deque
    from contextlib import ExitStack

    ctx = ExitStack()
    with ctx:
        sb = ctx.enter_context(tc.tile_pool(name="sb", bufs=1))
        pt_pool = ctx.enter_context(tc.tile_pool(name="pt", bufs=4))
        ot_pool = ctx.enter_context(tc.tile_pool(name="ot", bufs=3))
        rc_pool = ctx.enter_context(tc.tile_pool(name="rc", bufs=4))
        ysb_pool = ctx.enter_context(tc.tile_pool(name="ysb", bufs=3))
        ps_s = ctx.enter_context(tc.tile_pool(name="ps_s", bufs=2, space="PSUM"))
        ps_o = ctx.enter_context(tc.tile_pool(name="ps_o", bufs=1, space="PSUM"))
        ps_y = ctx.enter_context(tc.tile_pool(name="ps_y", bufs=2, space="PSUM"))

        # ---- persistent SBUF residents ----
        xT_s = sb.tile([128, CCH, N], BF, tag="xT")
        wq_s = sb.tile([128, CCH, 256], BF, tag="wq")
        wk_s = sb.tile([128, CCH, 256], BF, tag="wk")
        wv_s = sb.tile([128, CCH, 256], BF, tag="wv")
        wo_s = sb.tile([128, 2, DM], BF, tag="wo")
        tri_s = sb.tile([128, 128], BF, tag="tri")
        ones64 = sb.tile([1, 64], BF, tag="ones64")
        nc.vector.memset(ones64, 1.0)
        # per-n-chunk Q/K/V tiles so attention qc can start as soon as the
        # chunks it needs are projected (whole-tile dependency granularity)
        QT_t = [sb.tile([128, PAIRS, QC], BF, tag=f"QT{i}", name=f"QT{i}")
                for i in range(NQC)]
        KT_t = [sb.tile([128, PAIRS, QC], BF, tag=f"KT{i}", name=f"KT{i}")
                for i in range(NQC)]
        V1_t = [sb.tile([128, 4, HPC, 65], BF, tag=f"V1{i}", name=f"V1{i}")
                for i in range(NQC)]

        # DMA order: the HWDGE ring is FIFO — wq first, then x chunk 0 (so
        # the first projection starts ~4us in), then the rest interleaved.
        xT_r = xT.rearrange("(c p) n -> c p n", p=128)
        w_rs = [w_d.rearrange("(c p) m -> c p m", p=128)
                for w_d in (wqT, wkT, wvT)]
        w_ss = [wq_s, wk_s, wv_s]
        wo_r = woT.rearrange("(c p) j -> c p j", p=128)

        def dma_x(i):
            for c in range(CCH):
                nc.sync.dma_start(
                    out=xT_s[:, c, i * QC:(i + 1) * QC],
                    in_=xT_r[c][:, i * QC:(i + 1) * QC],
                )

        def dma_w(i):
            for c in range(CCH):
                nc.scalar.dma_start(out=w_ss[i][:, c, :], in_=w_rs[i][c])

        dma_w(0)
        dma_x(0)
        dma_w(1)
        dma_w(2)
        nc.scalar.dma_start(out=tri_s, in_=tri)
        dma_x(1)
        for c in range(2):
            nc.scalar.dma_start(out=wo_s[:, c, :], in_=wo_r[c])
        dma_x(2)
        dma_x(3)
        for i in range(NQC):
            nc.vector.memset(V1_t[i][:, :, :, 64], 1.0)

        # PE warm-up: ~25 dependency-free matmuls on a zeroed tile keep the
        # HAM activity window busy during the initial DMA wait, so the real
        # first matmuls run at the full 2.4 GHz clock.
        warm = sb.tile([128, 512], BF, tag="warm")
        nc.vector.memset(warm[:, 0:8], 0.0)
        pmW = ps_y.tile([128, QC], F32, tag="y", name="pmW")
        for i in range(20):
            nc.tensor.matmul(pmW[0:8, 0:256], warm[:, 0:8], warm[:, 0:256],
                             start=True, stop=True)

        exp = mybir.ActivationFunctionType.Exp

        # PE filler queue: projection / output-projection matmul groups are
        # drained one per kt-step inside the (ACT-bound) attention loop so
        # the PE never idles long enough for HAM to re-throttle its clock.
        fillers = deque()

        def drain(k=1, reserve=0):
            for _ in range(k):
                if len(fillers) > reserve:
                    fillers.popleft()()

        def proj_groups(nch):
            gs = []
            for w_s, t_s in ((wq_s, QT_t[nch]), (wk_s, KT_t[nch])):
                for pair in range(PAIRS):
                    def g(w_s=w_s, t_s=t_s, pair=pair, nch=nch):
                        pm = ps_y.tile([128, QC], F32, tag="y", name="pmqk")
                        for c in range(CCH):
                            nc.tensor.matmul(
                                pm,
                                w_s[:, c, pair * 128:(pair + 1) * 128],
                                xT_s[:, c, nch * QC:(nch + 1) * QC],
                                start=(c == 0),
                                stop=(c == CCH - 1),
                            )
                        nc.vector.tensor_copy(t_s[:, pair, :], pm)
                    gs.append(g)
            for sub in range(4):
                def g(sub=sub, nch=nch):
                    nt = nch * 4 + sub
                    pm = ps_y.tile([128, QC], F32, tag="y", name="pmv")
                    pmv = pm[:, 0:256]
                    for c in range(CCH):
                        nc.tensor.matmul(
                            pmv,
                            xT_s[:, c, nt * 128:(nt + 1) * 128],
                            wv_s[:, c, :],
                            start=(c == 0),
                            stop=(c == CCH - 1),
                        )
                    nc.vector.tensor_copy(
                        V1_t[nch][:, sub, :, 0:64],
                        pmv.rearrange("p (h d) -> p h d", h=HPC),
                    )
                gs.append(g)
            return gs

        def outproj_groups(qc, ot_tiles):
            gs = []
            ysbs = {}
            for qt in range(4):
                for jc in range(2):
                    def g(qt=qt, jc=jc, qc=qc, ot_tiles=ot_tiles):
                        if jc == 0:
                            ysbs[qt] = ysb_pool.tile(
                                [128, DM], F32, tag="ysb", name="ysb")
                        ysb = ysbs[qt]
                        pmY = ps_y.tile([128, QC], F32, tag="y", name="pmY")
                        for pair in range(PAIRS):
                            nc.tensor.matmul(
                                pmY,
                                ot_tiles[pair][:, qt * 128:(qt + 1) * 128],
                                wo_s[:, pair, jc * QC:(jc + 1) * QC],
                                start=(pair == 0),
                                stop=(pair == 1),
                            )
                        nc.vector.tensor_copy(
                            ysb[:, jc * QC:(jc + 1) * QC], pmY
                        )
                        if jc == 1:
                            r0 = qc * QC + qt * 128
                            nc.sync.dma_start(out=y[r0:r0 + 128, :], in_=ysb)
                    gs.append(g)
            return gs

        def attention(qc):
            ot_tiles = []
            for pair in range(PAIRS):
                psO = [
                    ps_o.tile([65, QC], F32, tag=f"o{h}", name=f"psO{h}")
                    for h in range(2)
                ]
                for kt in range(4 * (qc + 1)):
                    j = kt - 4 * qc          # >= 0 -> diagonal-band tile
                    q0 = max(0, j * 128)
                    nq = QC - q0
                    KTc = KT_t[kt // 4]
                    kk = (kt % 4) * 128
                    pmS = ps_s.tile([128, 1024], F32, tag="s", name="pmS")
                    for h in range(2):
                        nc.tensor.matmul(
                            pmS[:, h * QC + q0: (h + 1) * QC],
                            KTc[64 * h:64 * (h + 1), pair, kk:kk + 128],
                            QT_t[qc][64 * h:64 * (h + 1), pair, q0:QC],
                            start=True,
                            stop=True,
                        )
                    drain(1, reserve=4)
                    PT = pt_pool.tile([128, 1024], BF, tag="pt", name="PT")
                    if q0 == 0:
                        nc.scalar.activation(PT, pmS, exp, scale=0.125)
                    else:
                        pv = bass.AP(tensor=pmS.tensor, offset=pmS.offset + q0,
                                     ap=[pmS.ap[0], [QC, 2], [1, nq]])
                        tv = bass.AP(tensor=PT.tensor, offset=PT.offset + q0,
                                     ap=[PT.ap[0], [QC, 2], [1, nq]])
                        nc.scalar.activation(tv, pv, exp, scale=0.125)
                    if j >= 0:
                        PTm = pt_pool.tile([128, 256], BF, tag="ptm", name="PTm")
                        srcm = bass.AP(tensor=PT.tensor, offset=PT.offset + q0,
                                       ap=[PT.ap[0], [QC, 2], [1, 128]])
                        nc.vector.tensor_mul(
                            PTm.rearrange("p (a k) -> p a k", k=128),
                            srcm,
                            _free_repeat(tri_s, 2),
                        )
                    for h in range(2):
                        lhs = V1_t[kt // 4][:, kt % 4, pair * 2 + h, :]
                        if j >= 0:
                            nc.tensor.matmul(
                                psO[h][:, q0:q0 + 128],
                                lhs,
                                PTm[:, h * 128:(h + 1) * 128],
                                start=(kt == 0),
                                stop=(j == 3),
                            )
                            if q0 + 128 < QC:
                                nc.tensor.matmul(
                                    psO[h][:, q0 + 128:QC],
                                    lhs,
                                    PT[:, h * QC + q0 + 128:(h + 1) * QC],
                                    start=(kt == 0),
                                    stop=False,
                                )
                        else:
                            nc.tensor.matmul(
                                psO[h],
                                lhs,
                                PT[:, h * QC:(h + 1) * QC],
                                start=(kt == 0),
                                stop=False,
                            )
                # Free psO quickly (copy O_un + denom out); normalize
                # otp = O_un * (1/d) with 1/d broadcast across partitions
                # via a tiny ones-column matmul. Two fillers are drained
                # between the DVE chain and the psB matmuls so the PE has
                # work while the reciprocal completes.
                otp = ot_pool.tile([128, QC], BF, tag=f"ot{pair}", name="otp")
                otu = ot_pool.tile([128, QC], BF, tag=f"otu{pair}", name="otu")
                dn = rc_pool.tile([1, 1024], F32, tag="dn", name="dn")
                for h in range(2):
                    nc.vector.tensor_copy(
                        dn[:, h * QC:(h + 1) * QC], psO[h][64:65, :]
                    )
                rc = rc_pool.tile([1, 1024], F32, tag="rc", name="rc")
                nc.vector.reciprocal_approx_fast(rc, dn)
                rc16 = rc_pool.tile([1, 1024], BF, tag="rc16", name="rc16")
                nc.vector.tensor_copy(rc16, rc)
                for h in range(2):
                    nc.vector.tensor_copy(
                        otu[64 * h:64 * (h + 1), :], psO[h][0:64, :]
                    )
                drain(2)
                for h in range(2):
                    psB = ps_y.tile([64, QC], F32, tag="y", name="psB")
                    nc.tensor.matmul(
                        psB, ones64, rc16[0:1, h * QC:(h + 1) * QC],
                        start=True, stop=True,
                    )
                    nc.vector.tensor_mul(
                        otp[64 * h:64 * (h + 1), :],
                        otu[64 * h:64 * (h + 1), :],
                        psB,
                    )
                ot_tiles.append(otp)
            return ot_tiles

        # direct first projection, then attention chunks with fillers
        for g in proj_groups(0):
            g()
        prev_ot = None
        for nch in range(NQC):
            if nch + 1 < NQC:
                fillers.extend(proj_groups(nch + 1))
            if prev_ot is not None:
                fillers.extend(outproj_groups(nch - 1, prev_ot))
            prev_ot = attention(nch)
        while fillers:
            drain(1)
        for g in outproj_groups(NQC - 1, prev_ot):
            g()


_NC_CACHE = {}


def _get_program():
    if "nc" not in _NC_CACHE:
        _NC_CACHE["nc"] = build_program()
    return _NC_CACHE["nc"]


def kernel(x, Wq, bq, Wk, bk, Wv, bv, Wo):
    x = np.asarray(x, dtype=np.float32)
    Wq = np.asarray(Wq, dtype=np.float32)
    Wk = np.asarray(Wk, dtype=np.float32)
    Wv = np.asarray(Wv, dtype=np.float32)
    Wo = np.asarray(Wo, dtype=np.float32)

    nc = _get_program()
    tri = np.triu(np.ones((128, 128), dtype=np.float32)).astype(BF16)
    in_maps = []
    for core in range(NC_CORES):
        b, g = core // 4, core % 4
        hs = slice(256 * g, 256 * (g + 1))
        in_maps.append({
            "xT": np.ascontiguousarray(x[b].T).astype(BF16),
            "wqT": np.ascontiguousarray(Wq[hs].T).astype(BF16),
            "wkT": np.ascontiguousarray(Wk[hs].T).astype(BF16),
            "wvT": np.ascontiguousarray(Wv[hs].T).astype(BF16),
            "woT": np.ascontiguousarray(Wo[:, hs].T).astype(BF16),
            "tri": tri,
        })
    res = run_bass_kernel_spmd(nc, in_maps, list(range(NC_CORES)))
    out = np.zeros((BS, N, DM), dtype=np.float32)
    for core in range(NC_CORES):
        out[core // 4] += res.results[core]["y"]
    return out

